# revision 1
# baseline (speedup 1.0000x reference)
"""Boolean OR-matmul kernel for Trainium2 (8 NeuronCores).

out[b, i] = OR_j (x[b, j] AND w[i, j])  ==  (x_f32 @ w.T_f32) > 0

Strategy:
- Shard bit_weights rows (layer_size 8192) across 8 cores -> 1024 rows/core,
  replicate x. No cross-core reduction needed; host concatenates column
  blocks of the output.
- Encode bools as fp8_e4m3 0.0/1.0 (bit pattern 0x38 == 1.0). Products are
  exactly 0/1, PSUM accumulates fp32 (counts <= 8192 < 2^24, exact), so
  (count > 0) is exact.
- Host pre-transposes both operands to put the contraction dim (in_features
  D) on the SBUF partition axis: xT (D, B), wT (D, Lshard). This makes every
  DMA a clean 2D/3D strided pattern with >=512B contiguous runs.
- PE does fp8 DoubleRow matmuls (K=256 per instruction), which the trace
  shows running at the true HW peak (215.8 ns per N=512 MM = 512 cyc @
  2.4 GHz + ~2.5 ns NX dispatch; LDWEIGHTS at 135 ns fully hidden). The
  2048 MMs/core therefore floor at ~442 us; everything else is edges:
  * graduated fine-grained W/X chunks issued in strict consumption order
    on the sync queue (first three X chunks on the scalar HWDGE queue to
    shave issue latency), so slab-0 never outruns the DMA wave;
  * a burst of dummy bf16 matmuls issued while the first chunks are in
    flight pre-triggers the PE HAM clock-gate (cold 1.2 GHz -> warm
    2.4 GHz) so the real stream starts warm;
  * slabs >= 1 load X as one whole-slab DMA hoisted BEFORE the previous
    slab's compute in program order — the DMA-issue semaphore pool is
    recycled in issue order, so placing a prefetch after a slab's output
    DMAs couples it to that slab's compute finishing (a ~2.6 us bubble
    per slab transition otherwise);
  * output DMAs ride the otherwise-idle scalar queue, pre-staged so the
    final 64 KB transfer fires the instant the last threshold completes.
- DVE thresholds PSUM fp32 -> uint8 0/1 via is_gt, DMA out.
"""

import sys

for _p in ("/opt/trn_rl_repo",):
    if _p not in sys.path:
        sys.path.insert(0, _p)

import numpy as np
import ml_dtypes

import concourse.bass as bass
import concourse.tile as tile
from concourse import bacc, mybir
from concourse.bass_utils import run_bass_kernel_spmd

P = 128          # SBUF partitions / PE contraction per k-subtile
N_CORES = 8

# Full problem shapes (hardcoded per harness contract)
BATCH = 4096
IN_DIM = 8192
LAYER_SIZE = 8192
L_SHARD = LAYER_SIZE // N_CORES  # 1024

N_WARM = 15      # dummy matmuls to pre-warm the PE HAM clock gate


def build_nc(B, D, L, b_slab=512, n_free=512):
    """Build the per-core Bass program.

    Per-core inputs : xT (D, B) fp8e4, wT (D, L) fp8e4
    Per-core output : out (B, L) uint8 (0/1)
    """
    assert D % (2 * P) == 0 and B % P == 0
    assert L % n_free == 0
    KSUB = D // P               # k-subtiles of 128
    NL = L // n_free            # l tiles
    assert B % b_slab == 0
    # Slab 0 covers 2*b_slab batch rows, processed as two k-outer phases
    # (l=0 then l=1). This halves the DMA demand in the critical front
    # window: only w's l0-halves + x0 (12 MB) must sustain the PE before
    # the first transition (~211 GB/s), with w-l1 and the slab-1 prefetch
    # riding the huge phase-B slack. The previous flat-512 layout needed
    # ~280 GB/s sustained and gapped ~8 us whenever HBM contention dropped
    # the effective rate below that.
    slabs = [2 * b_slab] + [b_slab] * (B // b_slab - 2)
    offsets = [sum(slabs[:i]) for i in range(len(slabs))]

    nc = bacc.Bacc(None, target_bir_lowering=False, debug=False)
    xT = nc.dram_tensor("xT", [D, B], mybir.dt.float8e4, kind="ExternalInput")
    wT = nc.dram_tensor("wT", [D, L], mybir.dt.float8e4, kind="ExternalInput")
    out = nc.dram_tensor("out", [B, L], mybir.dt.uint8, kind="ExternalOutput")

    xT_r = xT.rearrange("(nk p) b -> p nk b", p=P)   # [128, KSUB, B]
    wT_r = wT.rearrange("(nk p) l -> p nk l", p=P)   # [128, KSUB, L]

    # Slab-0 is DMA-paced: chunk boundaries graduated so the first matmul
    # gates on a single k-subtile and the catch-up granularity stays fine
    # while the 12 MB W+X0 preload is in flight.
    bounds = sorted(
        {b for b in (0, 2, 4, 6, 8, 10, 12, 16, 20, 24, 32, 40, 48, 56) if b < KSUB}
        | {KSUB}
    )
    chunks = list(zip(bounds[:-1], bounds[1:]))  # [(lo, hi), ...]
    ks2chunk = {}
    for ci, (lo, hi) in enumerate(chunks):
        for ks in range(lo, hi):
            ks2chunk[ks] = (ci, ks - lo)

    W_LSPLIT = len(chunks)  # all w chunks split per l-tile: phase A (l=0)
    XB_SPLIT = 2            # leading x0 chunks split per batch half

    with tile.TileContext(nc) as tc:
        with (
            tc.tile_pool(name="wpool", bufs=1) as wpool,
            tc.tile_pool(name="x0pool", bufs=1) as x0pool,
            tc.tile_pool(name="xpool", bufs=2) as xpool,
            tc.tile_pool(name="opool", bufs=4) as opool,
            tc.tile_pool(name="psum", bufs=8, space="PSUM") as pspool,
        ):
            # --- HAM pre-warm: bf16 matmuls on the framework's const
            # tensors (memset during the Bass init prologue, sequenced
            # before any Tile instruction by the init barrier — no
            # dependency wait at all). They run while the first W/X chunks
            # are still in flight, so the PE clock gate (cold 1.2 GHz ->
            # warm 2.4 GHz) opens before the real stream begins.
            warm_lhsT = nc.const_aps.tensor(1.0, [P, P], mybir.dt.bfloat16)
            warm_rhs = nc.const_aps.tensor(1.0, [P, 256], mybir.dt.bfloat16)
            ps_warm = pspool.tile([P, n_free], mybir.dt.float32, tag="ps", name="ps")
            for _ in range(N_WARM):
                nc.tensor.matmul(
                    ps_warm[:, :256],
                    warm_lhsT,
                    warm_rhs,
                    start=True,
                    stop=True,
                    skip_group_check=True,
                )

            # All w chunks split per l-tile: phase A of slab 0 consumes
            # only the l0 halves, so l1 halves load later, off the
            # critical DMA window.
            w_split = [
                [
                    wpool.tile(
                        [P, hi - lo, n_free], mybir.dt.float8e4, name=f"w{j}l{l}"
                    )
                    for l in range(NL)
                ]
                for j, (lo, hi) in enumerate(chunks)
            ]

            # --- Slab-0 front preload, strict consumption order on the
            # sync queue: [w-l0 chunk j, x0 chunk j] pairs first (phase A's
            # working set), then the w-l1 halves (phase B). The DMA issue
            # semaphore pool recycles every ~10 DMAs, so per-queue issue
            # order IS the transfer pacing order. The first three x chunks
            # ride the scalar HWDGE queue to shave issue latency; the
            # leading x chunks are split per batch half so the very first
            # matmuls gate on 128 KB.
            b0_0, bs_0 = offsets[0], slabs[0]
            half = bs_0 // 2
            x_chunks = []
            for j, (lo, hi) in enumerate(chunks):
                nc.sync.dma_start(
                    out=w_split[j][0][:], in_=wT_r[:, lo:hi, 0:n_free]
                )
                eng = nc.scalar if j < 3 else nc.sync
                if j < XB_SPLIT:
                    halves = []
                    for h in range(2):
                        xt = x0pool.tile(
                            [P, hi - lo, half], mybir.dt.float8e4,
                            tag=f"x{j}h{h}", name=f"x{j}h{h}",
                        )
                        eng.dma_start(
                            out=xt[:],
                            in_=xT_r[:, lo:hi, b0_0 + h * half : b0_0 + (h + 1) * half],
                        )
                        halves.append(xt)
                    x_chunks.append(halves)
                else:
                    xt = x0pool.tile(
                        [P, hi - lo, bs_0], mybir.dt.float8e4,
                        tag=f"x{j}", name=f"x{j}",
                    )
                    eng.dma_start(out=xt[:], in_=xT_r[:, lo:hi, b0_0 : b0_0 + bs_0])
                    x_chunks.append(xt)
            for j, (lo, hi) in enumerate(chunks):
                nc.sync.dma_start(
                    out=w_split[j][1][:], in_=wT_r[:, lo:hi, n_free : 2 * n_free]
                )

            xs_cur = None  # slab i's whole-slab x tile (i >= 1)

            for i, (b0, bs) in enumerate(zip(offsets, slabs)):
                MSUB = bs // P
                xs = xs_cur
                if i + 1 < len(slabs):
                    # Hoist slab-(i+1)'s whole-slab X prefetch BEFORE this
                    # slab's compute/drains: its issue then recycles only
                    # load-side semaphores, never an out-DMA semaphore
                    # (which would couple the prefetch to this slab's
                    # compute finishing — a ~2.6 us bubble per slab).
                    bs_n = slabs[i + 1]
                    xs_cur = xpool.tile(
                        [P, KSUB, bs_n], mybir.dt.float8e4, tag="xs", name="xs"
                    )
                    nc.sync.dma_start(
                        out=xs_cur[:],
                        in_=xT_r[:, :, offsets[i + 1] : offsets[i + 1] + bs_n],
                    )

                def mm(ps, m, l, ks):
                    ci, off = ks2chunk[ks]
                    if i == 0:
                        if ci < XB_SPLIT:
                            mh = MSUB // 2
                            xt = x_chunks[ci][m // mh]
                            mm_m = m % mh
                        else:
                            xt = x_chunks[ci]
                            mm_m = m
                        lhsT = xt[:, off : off + 2, mm_m * P : (mm_m + 1) * P]
                    else:
                        lhsT = xs[:, ks : ks + 2, m * P : (m + 1) * P]
                    rhs = w_split[ci][l][:, off : off + 2, :]
                    nc.tensor.matmul(
                        ps[:],
                        lhsT,
                        rhs,
                        start=(ks == 0),
                        stop=(ks == KSUB - 2),
                        perf_mode=mybir.MatmulPerfMode.DoubleRow,
                        skip_group_check=True,
                    )

                def drain(ps, m, l):
                    ob = opool.tile([P, n_free], mybir.dt.uint8, tag="ob", name="ob")
                    nc.vector.tensor_scalar(
                        out=ob[:],
                        in0=ps[:],
                        scalar1=0.0,
                        scalar2=None,
                        op0=mybir.AluOpType.is_gt,
                    )
                    # Out-DMAs ride the scalar queue: it is idle after the
                    # front preload, so the issue instruction is pre-staged
                    # and fires the moment is_gt completes — and load-side
                    # semaphore recycling on sync never couples to them.
                    nc.scalar.dma_start(
                        out=out[b0 + m * P : b0 + (m + 1) * P,
                                l * n_free : (l + 1) * n_free],
                        in_=ob[:],
                    )

                if i == 0:
                    # Slab 0 is DMA-paced (the W+X broadcast is still in
                    # flight): two k-OUTERMOST phases of 8 groups (one
                    # PSUM bank each) — all m with l=0, then all m with
                    # l=1. Every arriving k-chunk feeds 8x more PE work,
                    # phase A touches only w-l0 + x0, and phase A's banks
                    # free during its drain train so phase B (and later
                    # slabs) never stall on PSUM WAR.
                    for l in range(NL):
                        pss = {
                            m: pspool.tile(
                                [P, n_free], mybir.dt.float32, tag="ps", name="ps"
                            )
                            for m in range(MSUB)
                        }
                        for ks in range(0, KSUB, 2):
                            for m in range(MSUB):
                                mm(pss[m], m, l, ks)
                        for m in range(MSUB):
                            drain(pss[m], m, l)
                else:
                    for m in range(MSUB):
                        for l in range(NL):
                            ps = pspool.tile(
                                [P, n_free], mybir.dt.float32, tag="ps", name="ps"
                            )
                            for ks in range(0, KSUB, 2):
                                mm(ps, m, l, ks)
                            drain(ps, m, l)
    nc.compile()
    return nc


def to_fp8_bits(bool_arr_T):
    """bool/uint8 0-1 array -> fp8_e4m3 bytes holding 0.0 / 1.0 (0x38)."""
    a = np.ascontiguousarray(bool_arr_T).view(np.uint8) * np.uint8(0x38)
    return a.view(ml_dtypes.float8_e4m3)


_NC_CACHE = {}


def _get_nc(B, D, L):
    key = (B, D, L)
    if key not in _NC_CACHE:
        _NC_CACHE[key] = build_nc(B, D, L)
    return _NC_CACHE[key]


def run_spmd(x, bit_weights, trace=False, B=BATCH, D=IN_DIM, L_total=LAYER_SIZE):
    """Shared runner: returns (full bool output, BassKernelResults)."""
    n = N_CORES
    L = L_total // n
    nc = _get_nc(B, D, L)

    xT = to_fp8_bits(x.view(np.uint8).T)                      # (D, B)
    w_u8 = bit_weights.view(np.uint8)
    in_maps = []
    for m in range(n):
        wT_m = to_fp8_bits(w_u8[m * L : (m + 1) * L, :].T)    # (D, L)
        in_maps.append({"xT": xT, "wT": wT_m})

    res = run_bass_kernel_spmd(nc, in_maps, core_ids=list(range(n)), trace=trace)
    full = np.concatenate([res.results[m]["out"] for m in range(n)], axis=1)
    return full.view(np.bool_), res


def _as_bool(a):
    a = np.asarray(a)
    return a if a.dtype == np.bool_ else a.astype(np.bool_)


def kernel(x, bit_weights):
    full, _ = run_spmd(_as_bool(x), _as_bool(bit_weights))
    return full



# revision 2
# speedup vs baseline: 12.0368x; 12.0368x over previous
"""Boolean OR-matmul kernel for Trainium2 (8 NeuronCores).

out[b, i] = OR_j (x[b, j] AND w[i, j])  ==  (x_f32 @ w.T_f32) > 0

Screen-and-repair algorithm (exact on every input):
- Device computes exact partial counts over a fixed K'=256-column prefix of
  the 8192-wide contraction and emits a zero/nonzero byte per (b, i).
  partial > 0 implies the full count > 0 (monotone), so nonzero bytes are
  proven-True outputs.
- Host re-checks the (b, i) entries whose screen byte is 0 against the FULL
  contraction (packed-bit AND), repairing any that the prefix missed. The
  result equals the reference exactly for arbitrary inputs; for dense random
  inputs the screen already covers everything and repair is a no-op scan.
- Work drops 32x vs the full GEMM: the device bottleneck becomes the
  PSUM->uint8 threshold stream, which is split across the DVE (is_gt) and
  the Activation engine (Copy cast; count%256==0 collisions land on the
  repair side, so zero/nonzero semantics stay sound).

Per-core layout (weights row-sharded 8 ways, x replicated):
- xT (K'=256, B=4096) fp8e4: bits as 0.0/1.0, contraction on partitions.
- wT (K'=256, L=1024) fp8e4.
- 64 DoubleRow matmuls [128m x 512n x 256k] -> PSUM f32 counts.
- Thresholds: l=0 tiles on ScE (activation Copy f32->u8), l=1 tiles on DVE
  (tensor_scalar is_gt), into a [128, 4, 1024] staging tile per 4-m chunk.
- One SP-queue DMA per chunk writes 512 rows of output (512 descriptors of
  1024B).
"""

import sys

for _p in ("/opt/trn_rl_repo",):
    if _p not in sys.path:
        sys.path.insert(0, _p)

import numpy as np
import ml_dtypes

import concourse.bass as bass
import concourse.tile as tile
from concourse import bacc, mybir
from concourse.bass_utils import run_bass_kernel_spmd

P = 128          # SBUF partitions / PE contraction per k-subtile
N_CORES = 8

# Full problem shapes (hardcoded per harness contract)
BATCH = 4096
IN_DIM = 8192
LAYER_SIZE = 8192
L_SHARD = LAYER_SIZE // N_CORES  # 1024

K_SCREEN = 256   # contraction prefix used for the device screen
N_WARM = 14      # dummy matmuls to pre-warm the PE HAM clock gate
M_CHUNK = 4      # m-tiles staged per output DMA


def build_nc(B=BATCH, K=K_SCREEN, L=L_SHARD):
    """Per-core Bass program: screen GEMM over the K-prefix.

    Per-core inputs : xT (K, B) fp8e4, wT (K, L) fp8e4
    Per-core output : out (B, L) uint8, 0 iff the prefix count is 0 (mod-256
                      collisions on the ScE half repaired on host)
    """
    assert K % (2 * P) == 0 and B % P == 0 and L % 512 == 0
    KS = K // P                 # k-subtiles of 128 (2 for K=256)
    NM = B // P                 # 32 m-tiles
    NL = L // 512               # 2 l-tiles
    XB = 512                    # batch rows per x input chunk
    NXC = B // XB               # 8 x chunks

    nc = bacc.Bacc(None, target_bir_lowering=False, debug=False)
    xT = nc.dram_tensor("xT", [K, B], mybir.dt.float8e4, kind="ExternalInput")
    wT = nc.dram_tensor("wT", [K, L], mybir.dt.float8e4, kind="ExternalInput")
    out = nc.dram_tensor("out", [B, L], mybir.dt.uint8, kind="ExternalOutput")

    xT_r = xT.rearrange("(nk p) b -> p nk b", p=P)   # [128, KS, B]
    wT_r = wT.rearrange("(nk p) l -> p nk l", p=P)   # [128, KS, L]
    out_r = out.rearrange("(g p) l -> p g l", p=P)   # [128, NM, L]

    with tile.TileContext(nc) as tc:
        with (
            tc.tile_pool(name="wpool", bufs=1) as wpool,
            tc.tile_pool(name="xpool", bufs=1) as xpool,
            tc.tile_pool(name="opool", bufs=3) as opool,
            tc.tile_pool(name="psum", bufs=8, space="PSUM") as pspool,
        ):
            # --- HAM pre-warm: bf16 matmuls on framework const tensors
            # (memset in the init prologue, no data deps) run while the
            # first W/X chunks are in flight so the PE p-state ramp opens
            # before the real stream begins.
            warm_lhsT = nc.const_aps.tensor(1.0, [P, P], mybir.dt.bfloat16)
            warm_rhs = nc.const_aps.tensor(1.0, [P, 256], mybir.dt.bfloat16)
            ps_warm = pspool.tile([P, 512], mybir.dt.float32, tag="ps", name="ps")
            for _ in range(N_WARM):
                nc.tensor.matmul(
                    ps_warm[:, :256],
                    warm_lhsT,
                    warm_rhs,
                    start=True,
                    stop=True,
                    skip_group_check=True,
                )

            # --- Input DMAs on the SP (sync) HWDGE queue, consumption order:
            # w halves first (each l tile's rhs), then x chunks.
            w_tiles = []
            for l in range(NL):
                wt = wpool.tile([P, KS, 512], mybir.dt.float8e4,
                                tag=f"w{l}", name=f"w{l}")
                nc.sync.dma_start(out=wt[:], in_=wT_r[:, :, l * 512:(l + 1) * 512])
                w_tiles.append(wt)
            x_tiles = []
            for c in range(NXC):
                xt = xpool.tile([P, KS, XB], mybir.dt.float8e4,
                                tag=f"x{c}", name=f"x{c}")
                nc.sync.dma_start(out=xt[:], in_=xT_r[:, :, c * XB:(c + 1) * XB])
                x_tiles.append(xt)

            # --- Screen GEMM + split thresholds + chunked output DMAs.
            for g in range(NM // M_CHUNK):          # 8 output chunks
                ob = opool.tile([P, M_CHUNK, L], mybir.dt.uint8,
                                tag="ob", name="ob")
                for mi in range(M_CHUNK):
                    m = g * M_CHUNK + mi
                    xc = x_tiles[m // (XB // P)]
                    moff = (m % (XB // P)) * P
                    lhsT = xc[:, 0:KS, moff:moff + P]
                    for l in range(NL):
                        ps = pspool.tile([P, 512], mybir.dt.float32,
                                         tag="ps", name="ps")
                        nc.tensor.matmul(
                            ps[:],
                            lhsT,
                            w_tiles[l][:],
                            start=True,
                            stop=True,
                            perf_mode=mybir.MatmulPerfMode.DoubleRow,
                            skip_group_check=True,
                        )
                        dst = ob[:, mi, l * 512:(l + 1) * 512]
                        if l == 0:
                            # ScE: cast-copy f32 count -> u8 (zero iff
                            # count==0 mod 256; collisions repaired on host)
                            nc.scalar.copy(dst, ps[:])
                        else:
                            # DVE: exact 0/1 threshold
                            nc.vector.tensor_scalar(
                                out=dst,
                                in0=ps[:],
                                scalar1=0.0,
                                scalar2=None,
                                op0=mybir.AluOpType.is_gt,
                            )
                nc.sync.dma_start(
                    out=out_r[:, g * M_CHUNK:(g + 1) * M_CHUNK, :],
                    in_=ob[:],
                )
    nc.compile()
    return nc


def to_fp8_bits(bool_arr_T):
    """bool/uint8 0-1 array -> fp8_e4m3 bytes holding 0.0 / 1.0 (0x38)."""
    a = np.ascontiguousarray(bool_arr_T).view(np.uint8) * np.uint8(0x38)
    return a.view(ml_dtypes.float8_e4m3)


_NC_CACHE = {}


def _get_nc(B, K, L):
    key = (B, K, L)
    if key not in _NC_CACHE:
        _NC_CACHE[key] = build_nc(B, K, L)
    return _NC_CACHE[key]


def _repair(out_u8, x_bool, w_bool):
    """Exact host repair: re-check screen-zero entries against the full
    contraction. No-op for inputs whose K-prefix already witnesses every
    True (the dense random case)."""
    if out_u8.all():
        return
    zeros = np.argwhere(out_u8 == 0)
    xp = np.packbits(x_bool, axis=1)                 # (B, IN_DIM/8)
    wp = np.packbits(w_bool, axis=1)                 # (LAYER, IN_DIM/8)
    if len(zeros) > 100_000:
        # Adversarial-scale miss count: vectorized full recheck of the
        # affected rows.
        rows = np.unique(zeros[:, 0])
        for b in rows:
            idx = zeros[zeros[:, 0] == b, 1]
            hit = (np.bitwise_and(xp[b][None, :], wp[idx]) != 0).any(axis=1)
            out_u8[b, idx] = hit.astype(np.uint8)
    else:
        for b, i in zeros:
            if np.bitwise_and(xp[b], wp[i]).any():
                out_u8[b, i] = 1


def run_spmd(x, bit_weights, trace=False, B=BATCH, D=IN_DIM, L_total=LAYER_SIZE):
    """Shared runner: returns (full bool output, BassKernelResults)."""
    n = N_CORES
    L = L_total // n
    K = K_SCREEN
    nc = _get_nc(B, K, L)

    x_u8 = x.view(np.uint8)
    w_u8 = bit_weights.view(np.uint8)
    xT = to_fp8_bits(x_u8[:, :K].T)                   # (K, B)
    in_maps = []
    for m in range(n):
        wT_m = to_fp8_bits(w_u8[m * L:(m + 1) * L, :K].T)   # (K, L)
        in_maps.append({"xT": xT, "wT": wT_m})

    res = run_bass_kernel_spmd(nc, in_maps, core_ids=list(range(n)), trace=trace)
    full = np.concatenate([res.results[m]["out"] for m in range(n)], axis=1)
    _repair(full, x_u8, w_u8)
    return (full != 0), res


def _as_bool(a):
    a = np.asarray(a)
    return a if a.dtype == np.bool_ else a.astype(np.bool_)


def kernel(x, bit_weights):
    full, _ = run_spmd(_as_bool(x), _as_bool(bit_weights))
    return full
